# revision 1
# baseline (speedup 1.0000x reference)
import numpy as np
import jax
import jax.numpy as jnp
from functools import partial

# nn_AttentionSequencePoolingLayer: B=2048, L=200, D=128, H1=128, H2=32
# Sharding: pure data parallel over B across 8 NeuronCores; MLP weights
# replicated. Global Dice batch-norm statistics via cross-core psum.

EPS = 1e-8
B, L, D = 2048, 200, 128
NC = 8
BS = B // NC  # 256 rows per core


def _dice(x, alpha, m, v):
    p = jax.nn.sigmoid((x - m) * jax.lax.rsqrt(v + EPS))
    return p * x + (1.0 - p) * alpha * x


@partial(jax.pmap, axis_name="i")
def _run(query, keys, keys_len, W1, b1, alpha1, W2, b2, alpha2, Wd, bd):
    n = float(B * L)
    q = jnp.broadcast_to(query[:, None, :], (BS, L, D))
    x = jnp.concatenate([q, keys, q - keys, q * keys], axis=-1)
    a1 = x @ W1 + b1
    s1 = jax.lax.psum(jnp.sum(a1, axis=(0, 1)), "i")
    ss1 = jax.lax.psum(jnp.sum(a1 * a1, axis=(0, 1)), "i")
    m1 = s1 / n
    v1 = ss1 / n - m1 * m1
    h = _dice(a1, alpha1, m1, v1)
    a2 = h @ W2 + b2
    s2 = jax.lax.psum(jnp.sum(a2, axis=(0, 1)), "i")
    ss2 = jax.lax.psum(jnp.sum(a2 * a2, axis=(0, 1)), "i")
    m2 = s2 / n
    v2 = ss2 / n - m2 * m2
    h2 = _dice(a2, alpha2, m2, v2)
    score = (h2 @ Wd + bd)[..., 0]
    mask = jnp.arange(L) < keys_len[:, None]
    score = jnp.where(mask, score, 0.0)
    return jnp.einsum("bl,bld->bd", score, keys)


def kernel(query, keys, keys_len, W1, b1, alpha1, W2, b2, alpha2, Wd, bd):
    out_dtype = np.asarray(query).dtype
    q = np.asarray(query, np.float32).reshape(NC, BS, D)
    k = np.asarray(keys, np.float32).reshape(NC, BS, L, D)
    kl = np.asarray(keys_len, np.int32).reshape(NC, BS)

    def rep(w):
        w = np.asarray(w, np.float32)
        return np.broadcast_to(w, (NC,) + w.shape)

    out = _run(q, k, kl, rep(W1), rep(b1), rep(alpha1),
               rep(W2), rep(b2), rep(alpha2), rep(Wd), rep(bd))
    return np.asarray(out).reshape(B, D).astype(out_dtype, copy=False)


# revision 2
# speedup vs baseline: 1.0403x; 1.0403x over previous
import numpy as np
import jax
import jax.numpy as jnp
from functools import partial

# nn_AttentionSequencePoolingLayer: B=2048, L=200, D=128, H1=128, H2=32
# Sharding: pure data parallel over B across 8 NeuronCores; MLP weights
# replicated. Global Dice batch-norm statistics via cross-core psum.

EPS = 1e-8
B, L, D = 2048, 200, 128
NC = 8
BS = B // NC  # 256 rows per core


def _dice(x, alpha, m, v):
    p = jax.nn.sigmoid((x - m) * jax.lax.rsqrt(v + EPS))
    return p * x + (1.0 - p) * alpha * x


@partial(jax.pmap, axis_name="i")
def _run(query, keys, keys_len, W1, b1, alpha1, W2, b2, alpha2, Wd, bd):
    n = float(B * L)
    # x = [q, k, q-k, q*k] @ W1 rewritten to avoid materializing (BS, L, 4D):
    # a1 = q@(W1a+W1c) + k@(W1b-W1c) + (q*k)@W1d + b1
    W1a, W1b, W1c, W1d = W1[:D], W1[D:2 * D], W1[2 * D:3 * D], W1[3 * D:]
    cq = query @ (W1a + W1c) + b1                     # (BS, H1) per-row const
    a1 = cq[:, None, :] + keys @ (W1b - W1c) + (query[:, None, :] * keys) @ W1d
    s1 = jax.lax.psum(jnp.sum(a1, axis=(0, 1)), "i")
    ss1 = jax.lax.psum(jnp.sum(a1 * a1, axis=(0, 1)), "i")
    m1 = s1 / n
    v1 = ss1 / n - m1 * m1
    h = _dice(a1, alpha1, m1, v1)
    a2 = h @ W2 + b2
    s2 = jax.lax.psum(jnp.sum(a2, axis=(0, 1)), "i")
    ss2 = jax.lax.psum(jnp.sum(a2 * a2, axis=(0, 1)), "i")
    m2 = s2 / n
    v2 = ss2 / n - m2 * m2
    h2 = _dice(a2, alpha2, m2, v2)
    score = (h2 @ Wd + bd)[..., 0]
    mask = jnp.arange(L) < keys_len[:, None]
    score = jnp.where(mask, score, 0.0)
    return jnp.einsum("bl,bld->bd", score, keys)


def kernel(query, keys, keys_len, W1, b1, alpha1, W2, b2, alpha2, Wd, bd):
    out_dtype = np.asarray(query).dtype
    q = np.asarray(query, np.float32).reshape(NC, BS, D)
    k = np.asarray(keys, np.float32).reshape(NC, BS, L, D)
    kl = np.asarray(keys_len, np.int32).reshape(NC, BS)

    def rep(w):
        w = np.asarray(w, np.float32)
        return np.broadcast_to(w, (NC,) + w.shape)

    out = _run(q, k, kl, rep(W1), rep(b1), rep(alpha1),
               rep(W2), rep(b2), rep(alpha2), rep(Wd), rep(bd))
    return np.asarray(out).reshape(B, D).astype(out_dtype, copy=False)
